# revision 36
# baseline (speedup 1.0000x reference)
"""Trainium2 Bass kernel for nn_AttentionBlock (GroupNorm + single-head
self-attention + projection + residual), x [4, 512, 64, 64] f32.

Sharding (8 NeuronCores, no collectives): core i takes batch b=i//2 and
query-half h=i%2 (2048 of the 4096 spatial positions).  Each core computes
full K/V for its batch element (duplicated across the pair), Q only for its
half, attention over all 4096 keys, projection and residual for its half.
The host shards inputs / gathers outputs.

Numerics: matmul operands are bf16 (x, weights, k, v, q, exp(S), O); all
accumulation is fp32 in PSUM, GroupNorm statistics / softmax denominators /
residual stay fp32.  bf16 operands enable Fast Weight Load (fp32/fp32r
weights disable FWL), which hides the per-matmul LDWEIGHTS cost.

Per-core structure (n=4096, nq=2048, c=512):
  GroupNorm is folded into the qkv weights (W' = W * scale_c): x feeds the
  matmuls directly.  Bias folding exploits softmax algebra: the K bias is
  dropped entirely (a per-query constant shift of the scores cancels in
  softmax), and the V bias is applied once at the end as a projected row
  broadcast (bv'^T Wp added into the residual tiles) instead of per V tile.
  Group statistics: PE computes group sums of x (and of x^2, via an ACT
  Square pass into scratch) with [8,512] membership matmuls as each x
  tile's DMA lands, with the per-tile PSUM reduce consumed immediately
  (a deferred consumer lets the PSUM ring re-issue the bank and races
  PE-write vs DVE-read -- a fatal collision on HW).  One short sqrt chain
  then yields all per-channel scales at once.  (A fused DVE
  tensor_tensor_reduce for the x^2 sums passes CoreSim but reliably
  kills the device -- do not reintroduce it.)
  Everything stays SBUF-resident in bf16: k [c, n] (32KB/part), v^T [n, c]
  (32KB/part), q [c, nq] (16KB/part) -- no DRAM spills or reloads.
  Attention per (q-chunk of 512, key-chunk j of 128):
    S^T [j:128, q:512] = k^T q in PSUM, E = exp(S^T/sqrt(c)) -> bf16,
    O[co, q] += v^T_j[:, co]^T E (PE), esum[p, q] += E (DVE).
  The softmax denominator is ONE matmul per q-chunk (1^T esum).  Then
  y^T [q:128, c:512] = (O chunk)^T Wp^T, scaled by 1/denom (transposed to
  [q, 1] via a tiny PE transpose) plus the residual (x^T + b_proj + bv'Wp).
  x DMA uses 8 parallel eighth-tile chains (tile-major; one DMA queue
  alone sustains only ~60-90 GB/s) and the weight DMAs are explicitly
  sequenced behind it so x owns the head's HBM bandwidth; bf16 "burst"
  matmuls keep the PE activity monitor at the fast clock through the
  DMA-bound head, the qkv->attention pool transition, and the final
  proj drains.
"""

import os
import numpy as np

B, C, HH, WW = 4, 512, 64, 64
N = HH * WW            # 4096
NQ = N // 2            # 2048 queries per core
NCORES = 8
CT = C // 128          # 4 channel tiles
PT = N // 512          # 8 spatial chunks of 512
QT = NQ // 512         # 4 query chunks of 512
JT = N // 128          # 32 key chunks of 128
GSIZE = 16             # channels per group
EPS = 1e-5
SCALE = 1.0 / float(np.sqrt(C))

_PROG = None


def _build_program():
    import concourse.bacc as bacc
    import concourse.tile as tile
    from concourse import mybir
    from contextlib import ExitStack

    F32 = mybir.dt.float32
    F32R = mybir.dt.float32r
    BF16 = mybir.dt.bfloat16

    nc = bacc.Bacc("TRN2", target_bir_lowering=False, debug=False,
                   num_devices=NCORES)

    def din(name, shape, dt):
        return nc.dram_tensor(name, shape, dt, kind="ExternalInput").ap()

    x_cn = din("x_cn", [C, N], BF16)    # x for this batch, query-half first
    xb_t = din("xb_t", [NQ, C], F32)    # x^T residual slice + b_proj
    w_qT = din("w_qT", [C, C], BF16)
    w_kT = din("w_kT", [C, C], BF16)
    w_vT = din("w_vT", [C, C], BF16)
    w_pT = din("w_pT", [C, C], BF16)
    b_v = din("b_v", [1, C], F32)
    cols = din("cols", [128, 3 * CT], F32)  # per c-tile: gamma, beta, b_q
    gmat = din("gmat", [128, 8], BF16)      # group membership (p//16 == u)
    gmat_f = din("gmat_f", [128, 8], F32)
    gmat_t = din("gmat_t", [8, 128], F32)
    y_t = nc.dram_tensor("y_t", [NQ, C], F32, kind="ExternalOutput").ap()

    AF = mybir.ActivationFunctionType
    OP = mybir.AluOpType

    with tile.TileContext(nc) as tc, ExitStack() as ctx:
        persist = ctx.enter_context(tc.tile_pool(name="persist", bufs=1))
        kpool = ctx.enter_context(tc.tile_pool(name="kpool", bufs=1))
        vqpool = ctx.enter_context(tc.tile_pool(name="vqpool", bufs=1))

        # ---- persistent small constants ----
        gma = persist.tile([128, 8], BF16)
        nc.sync.dma_start(out=gma, in_=gmat)
        gma_f = persist.tile([128, 8], F32)
        nc.sync.dma_start(out=gma_f, in_=gmat_f)
        gmt = persist.tile([8, 128], F32)
        nc.sync.dma_start(out=gmt, in_=gmat_t)
        one1 = persist.tile([1, 1], F32)
        nc.vector.memset(one1, 1.0)
        ones_row = persist.tile([1, 128], BF16)
        ones_st_r = persist.tile([1, 128], F32)
        nc.vector.memset(ones_st_r, 1.0)
        nc.vector.tensor_copy(ones_row, ones_st_r)
        ones_col = persist.tile([128, 1], F32R)
        ones_st_c = persist.tile([128, 1], F32)
        nc.vector.memset(ones_st_c, 1.0)
        nc.vector.tensor_copy(ones_col, ones_st_c)
        eps8 = persist.tile([8, 1], F32)
        nc.vector.memset(eps8, EPS)
        warm_a = persist.tile([128, 128], BF16)
        nc.vector.memset(warm_a, 0.03)
        warm_b = persist.tile([128, 512], BF16)
        nc.vector.memset(warm_b, 0.01)
        from concourse.bass import _add_dep_helper

        def emit_burst(wppool, dep_inst, n, nm, pstag="g", pbufs=3):
            # Dense bf16 matmuls paced by an explicit dep on dep_inst: trips
            # the PE activity monitor into the fast-clock state right where
            # it is needed (the DMA-bound head would otherwise let it lapse).
            if n <= 0:
                return
            wps = wppool.tile([128, 512], F32, tag=pstag,
                              name=f"wps_{nm}", bufs=pbufs)
            for wi in range(n):
                mm_i = nc.tensor.matmul(wps, warm_a, warm_b,
                                        start=(wi == 0), stop=(wi == n - 1))
                if wi == 0 and dep_inst is not None:
                    _add_dep_helper(mm_i.ins, dep_inst.ins, sync=True,
                                    reason="pace warm burst")
        cols_t = persist.tile([128, 3 * CT], F32)
        nc.sync.dma_start(out=cols_t, in_=cols)
        gcol = [cols_t[:, 3 * t:3 * t + 1] for t in range(CT)]
        bcol = [cols_t[:, 3 * t + 1:3 * t + 2] for t in range(CT)]
        bqcol = [cols_t[:, 3 * t + 2:3 * t + 3] for t in range(CT)]
        bvr = persist.tile([1, C], F32)
        wp_big = persist.tile([128, CT, C], BF16)
        wp = [wp_big[:, t, :] for t in range(CT)]
        bvp_sb = persist.tile([128, C], F32)   # broadcast bv'^T Wp row

        # resident for the whole kernel (bf16): k [c,n], v^T [n,c], q [c,nq]
        k_tiles = [kpool.tile([128, N], BF16, name=f"k_{t}", tag=f"k{t}")
                   for t in range(CT)]
        vt_big = vqpool.tile([128, JT, C], BF16, name="vt", tag="vt")
        q_big = vqpool.tile([128, CT, NQ], BF16, name="q", tag="q")

        with tc.tile_pool(name="xpool", bufs=1) as xpool, \
             tc.tile_pool(name="wmat", bufs=1) as wmat, \
             tc.tile_pool(name="gnsb", bufs=2) as gnsb, \
             tc.tile_pool(name="qps", bufs=1, space="PSUM") as qps:

            x_tiles = [xpool.tile([128, N], BF16, name=f"x_{t}", tag=f"x{t}")
                       for t in range(CT)]
            # x DMA: 8 parallel eighth-tile chains, tile-major -- tile t's
            # eight chunks land together, tiles arrive in order, and the
            # chains saturate the core's HBM bandwidth (one DMA queue alone
            # sustains only ~60-90 GB/s).
            prev_q = [None] * 8
            x_dmas = []
            for t in range(CT):
                for qq in range(8):
                    dma_i = nc.sync.dma_start(
                        out=x_tiles[t][:, qq * 512:(qq + 1) * 512],
                        in_=x_cn[t * 128:(t + 1) * 128,
                                 qq * 512:(qq + 1) * 512])
                    if prev_q[qq] is not None:
                        _add_dep_helper(dma_i.ins, prev_q[qq].ins,
                                        sync=True,
                                        reason="serialize x tile loads")
                    prev_q[qq] = dma_i
                x_dmas.append(dma_i)

            def load_w(srcw, nm, after=None):
                w_big = wmat.tile([128, CT, C], BF16, name=f"{nm}_big",
                                  tag="w", bufs=2)
                dma_i = nc.sync.dma_start(
                    out=w_big,
                    in_=srcw.rearrange("(t p) o -> p t o", t=CT))
                if after is not None:
                    _add_dep_helper(dma_i.ins, after.ins, sync=True,
                                    reason="keep weights off head DMA bw")
                return [w_big[:, t, :] for t in range(CT)], dma_i

            # wv needed ~20us in; everything else is sequenced later
            wv, wv_dma = load_w(w_vT, "wv", after=x_dmas[1])
            nc.sync.dma_start(out=bvr, in_=b_v)

            # ---------------- GroupNorm statistics ----------------
            # PE: per-tile group sums of x and x^2 via the [8,512]
            # membership matmuls, gx2 chained right behind gx so the PE
            # never drains between them.  x^2 comes from chunked DVE
            # tensor_mul passes into the k tile (overwritten later) -- the
            # squares land before gx finishes, so gx2 follows seamlessly.
            # ACT stays free for the stats chain.
            # NOTE: a fused DVE tensor_tensor_reduce for x^2 passes CoreSim
            # but reliably kills the device -- plain tensor_mul only.
            emit_burst(qps, None, 5, "init")
            stato = gnsb.tile([8, 8], F32, tag="stato", bufs=1)
            BRIDGE = (0, 2, 2, 2)
            for t in range(CT):
                if t > 0:
                    emit_burst(qps, x_dmas[t - 1], BRIDGE[t], f"br{t}")
                for sc4 in range(4):
                    nc.vector.tensor_mul(
                        k_tiles[t][:, sc4 * 1024:(sc4 + 1) * 1024],
                        x_tiles[t][:, sc4 * 1024:(sc4 + 1) * 1024],
                        x_tiles[t][:, sc4 * 1024:(sc4 + 1) * 1024])
                gx = qps.tile([8, 512], F32, tag="g", bufs=3,
                              name=f"gx{t}")
                for pc in range(PT):
                    nc.tensor.matmul(gx, gma,
                                     x_tiles[t][:, pc * 512:(pc + 1) * 512],
                                     start=(pc == 0), stop=(pc == PT - 1))
                # consume the PSUM tile IMMEDIATELY: a later emission would
                # let the bufs ring re-issue this bank to another matmul
                # before the reduce is known, racing PE-write vs DVE-read
                # (fatal PSUM collision on HW)
                nc.vector.reduce_sum(out=stato[:, t:t + 1], in_=gx,
                                     axis=mybir.AxisListType.X)
                gx2 = qps.tile([8, 512], F32, tag="gg", bufs=2,
                               name=f"gx2_{t}")
                for pc in range(PT):
                    nc.tensor.matmul(gx2, gma,
                                     k_tiles[t][:, pc * 512:(pc + 1) * 512],
                                     start=(pc == 0), stop=(pc == PT - 1))
                nc.vector.reduce_sum(out=stato[:, 4 + t:5 + t], in_=gx2,
                                     axis=mybir.AxisListType.X)
            # fill the stats-chain latency with warm matmuls
            emit_burst(qps, x_dmas[3], 6, "tail")
            grp = gnsb.tile([8, 8], F32, tag="grp", bufs=1)
            nc.scalar.mul(out=grp, in_=stato, mul=1.0 / (GSIZE * N))
            gm2 = gnsb.tile([8, 4], F32, tag="gm2", bufs=1)
            nc.vector.tensor_mul(gm2, grp[:, 0:4], grp[:, 0:4])
            var = gnsb.tile([8, 4], F32, tag="var", bufs=1)
            nc.vector.tensor_sub(var, grp[:, 4:8], gm2)
            std = gnsb.tile([8, 4], F32, tag="std", bufs=1)
            nc.scalar.activation(out=std, in_=var, func=AF.Sqrt,
                                 bias=eps8, scale=1.0)
            gout = gnsb.tile([8, 8], F32, tag="gout", bufs=1)
            nc.vector.reciprocal(out=gout[:, 0:4], in_=std)
            nc.vector.tensor_mul(gout[:, 4:8], grp[:, 0:4], gout[:, 0:4])
            # expand per-group [rstd | mean*rstd] to per-channel [128, 8]
            pg_ps = qps.tile([128, 8], F32, tag="g", bufs=3, name="pg")
            nc.tensor.matmul(pg_ps, gmt, gout, start=True, stop=True)
            pg_sb = gnsb.tile([128, 8], F32, tag="pg", bufs=1)
            nc.scalar.copy(out=pg_sb, in_=pg_ps)

            sc_f = []
            bct = []
            for t in range(CT):
                # per-channel scale = gamma*rstd ; bias = beta - mean*scale
                sc_t = gnsb.tile([128, 1], F32, tag=f"sc{t}", bufs=1)
                nc.vector.tensor_mul(sc_t, gcol[t], pg_sb[:, t:t + 1])
                sc_f.append(sc_t)
                nc.vector.tensor_scalar_mul(out=wv[t], in0=wv[t],
                                            scalar1=sc_t)
                msc = gnsb.tile([128, 1], F32, tag="msc")
                nc.vector.tensor_mul(msc, gcol[t], pg_sb[:, 4 + t:5 + t])
                bc_t = gnsb.tile([128, 1], F32, tag="bc")
                nc.vector.tensor_sub(bc_t, bcol[t], msc)
                rsc = gnsb.tile([128, 1], F32, tag="rsc")
                nc.vector.reciprocal(out=rsc, in_=sc_t)
                bct_t = gnsb.tile([128, 1], BF16, tag=f"bct{t}", bufs=1)
                nc.vector.tensor_mul(bct_t, bc_t, rsc)
                bct.append(bct_t)

            # weight-bias matvecs:  row_m = sum_c (bc/sc)_c^T (W ∘ sc)_c
            def bias_row(tiles, nm):
                row_ps = qps.tile([1, C], F32, tag="g", bufs=3,
                                  name=f"brow_{nm}")
                for c in range(CT):
                    nc.tensor.matmul(row_ps, bct[c], tiles[c],
                                     start=(c == 0), stop=(c == CT - 1))
                row_sb = gnsb.tile([1, C], F32, tag=f"brs_{nm}", bufs=1)
                nc.scalar.copy(out=row_sb, in_=row_ps)
                return row_sb

            def scale_w(tiles):
                for t in range(CT):
                    nc.vector.tensor_scalar_mul(out=tiles[t], in0=tiles[t],
                                                scalar1=sc_f[t])

            # ---------------- QKV ----------------
            # v^T = x^T Wv' : 32 resident tiles [128p, 512c] (bias folded
            # into the proj-end broadcast instead)
            for p in range(JT):
                vt_ps = qps.tile([128, C], F32, tag="mm", bufs=3)
                for c in range(CT):
                    nc.tensor.matmul(vt_ps,
                                     x_tiles[c][:, p * 128:(p + 1) * 128],
                                     wv[c], start=(c == 0),
                                     stop=(c == CT - 1))
                if p % 2 == 0:
                    nc.vector.tensor_copy(vt_big[:, p, :], vt_ps)
                else:
                    nc.scalar.copy(out=vt_big[:, p, :], in_=vt_ps)

            # ---- v/proj bias part 1 (before wk reuses wv's buffer slot):
            # out += bv' per channel before proj  ==  += (bv'^T Wp) after,
            # so fold bv' = Wv bc + b_v into one broadcast row added to the
            # residual tiles.
            vrow = bias_row(wv, "v")
            bvr_tot = gnsb.tile([1, C], F32, tag="bvrt", bufs=1)
            nc.vector.tensor_add(bvr_tot, vrow, bvr)
            bvcb = []
            for t in range(CT):
                bvc_ps = qps.tile([128, 1], F32, tag="g", bufs=3,
                                  name=f"bvc{t}")
                nc.tensor.transpose(bvc_ps,
                                    bvr_tot[0:1, t * 128:(t + 1) * 128],
                                    one1)
                bvc_t = gnsb.tile([128, 1], BF16, tag=f"bvcb{t}", bufs=1)
                nc.vector.tensor_copy(bvc_t, bvc_ps)
                bvcb.append(bvc_t)

            # q weights + bias columns (kept: the q bias shifts scores per
            # key, which does NOT cancel in softmax)
            wq_l, wq_dma = load_w(w_qT, "wq", after=x_dmas[3])
            scale_w(wq_l)
            qrow = bias_row(wq_l, "q")
            bq_tot = []
            for o in range(CT):
                bt_ps = qps.tile([128, 1], F32, tag="g", bufs=3,
                                 name=f"bt_q{o}")
                nc.tensor.transpose(bt_ps, qrow[0:1, o * 128:(o + 1) * 128],
                                    one1)
                tot = gnsb.tile([128, 1], F32, tag=f"btot_q{o}", bufs=1)
                nc.vector.tensor_add(tot, bt_ps, bqcol[o])
                bq_tot.append(tot)

            # q = Wq'^T x + bq' : resident [c, nq]; p-major so the first
            # 512 query columns are ready first
            for p in range(QT):
                for o in range(CT):
                    q_ps = qps.tile([128, 512], F32, tag="mm", bufs=3)
                    for c in range(CT):
                        nc.tensor.matmul(q_ps,
                                         wq_l[c][:, o * 128:(o + 1) * 128],
                                         x_tiles[c][:, p * 512:(p + 1) * 512],
                                         start=(c == 0), stop=(c == CT - 1))
                    if o % 2 == 0:
                        nc.vector.tensor_scalar_add(
                            out=q_big[:, o, p * 512:(p + 1) * 512],
                            in0=q_ps, scalar1=bq_tot[o])
                    else:
                        # same affine on ACT (Identity allows an AP bias)
                        nc.scalar.activation(
                            out=q_big[:, o, p * 512:(p + 1) * 512],
                            in_=q_ps, func=AF.Identity,
                            bias=bq_tot[o], scale=1.0)

            # k = Wk'^T x : resident [c,n] tiles.  NO bias: a k-bias shifts
            # every score of a query by the same constant, which softmax
            # normalizes away exactly.
            wk_l, wk_dma = load_w(w_kT, "wk", after=wq_dma)
            scale_w(wk_l)
            wp_dma = nc.sync.dma_start(
                out=wp_big, in_=w_pT.rearrange("(t p) o -> p t o", t=CT))
            _add_dep_helper(wp_dma.ins, wk_dma.ins, sync=True,
                            reason="keep wp off head DMA bw")
            for o in range(CT):
                for p in range(PT):
                    k_ps = qps.tile([128, 512], F32, tag="mm", bufs=3)
                    for c in range(CT):
                        nc.tensor.matmul(k_ps,
                                         wk_l[c][:, o * 128:(o + 1) * 128],
                                         x_tiles[c][:, p * 512:(p + 1) * 512],
                                         start=(c == 0), stop=(c == CT - 1))
                    if (o + p) % 2 == 0:
                        nc.vector.tensor_copy(
                            k_tiles[o][:, p * 512:(p + 1) * 512], k_ps)
                    else:
                        nc.scalar.copy(
                            out=k_tiles[o][:, p * 512:(p + 1) * 512],
                            in_=k_ps)

            # ---- v/proj bias part 2, emitted after the k matmuls so the
            # wp-gated matmuls never block the big phases.
            bvp_row_ps = qps.tile([1, C], F32, tag="g", bufs=3, name="bvpr")
            for t in range(CT):
                nc.tensor.matmul(bvp_row_ps, bvcb[t], wp[t],
                                 start=(t == 0), stop=(t == CT - 1))
            bvp_row = gnsb.tile([1, C], BF16, tag="bvprow", bufs=1)
            nc.scalar.copy(out=bvp_row, in_=bvp_row_ps)
            bvp_ps = qps.tile([128, C], F32, tag="g", bufs=3, name="bvpb")
            nc.tensor.matmul(bvp_ps, ones_row, bvp_row, start=True, stop=True)
            nc.scalar.copy(out=bvp_sb, in_=bvp_ps)
            # bridge the qkv->attention transition: the first s_ps tile
            # waits on this pool's address-zone release (~2us of PE idle
            # that would re-throttle the clock right as attention starts)
            emit_burst(qps, None, 8, "kab")

        # ---------------- attention + proj (per 512-wide q-chunk) -----------
        with tc.tile_pool(name="estream", bufs=4) as epool, \
             tc.tile_pool(name="esum", bufs=2) as espool, \
             tc.tile_pool(name="osb", bufs=2) as opool, \
             tc.tile_pool(name="ysb", bufs=2) as ypool, \
             tc.tile_pool(name="xbst", bufs=3) as xbpool, \
             tc.tile_pool(name="dsb", bufs=2) as dpool, \
             tc.tile_pool(name="psS", bufs=2, space="PSUM") as psS, \
             tc.tile_pool(name="psO", bufs=1, space="PSUM") as psO, \
             tc.tile_pool(name="psD", bufs=1, space="PSUM") as psD, \
             tc.tile_pool(name="psY", bufs=1, space="PSUM") as psY:

            for qc in range(QT):
                qa = [q_big[:, c, qc * 512:(qc + 1) * 512] for c in range(CT)]

                o_ps = [psO.tile([128, 512], F32, name=f"o_ps{co}",
                                 tag=f"o{co}") for co in range(CT)]
                esum = espool.tile([128, 512], F32R, tag="esum")

                def s_exp(j):
                    s_ps = psS.tile([128, 512], F32, tag="s")
                    for c in range(CT):
                        nc.tensor.matmul(s_ps,
                                         k_tiles[c][:, j * 128:(j + 1) * 128],
                                         qa[c], start=(c == 0),
                                         stop=(c == CT - 1))
                    e_sb = epool.tile([128, 512], BF16, tag="e")
                    nc.scalar.activation(out=e_sb, in_=s_ps, func=AF.Exp,
                                         scale=SCALE)
                    return e_sb

                # keep TWO exp tiles in flight: o(j) needs exp(j), which
                # lags the s(j) matmul group by ~0.9us on ACT; one s-group
                # of lookahead (~0.87us) leaves a ~70ns bubble every j, two
                # groups hide it completely
                e_pend = [s_exp(0), s_exp(1)]
                for j in range(JT):
                    if j + 2 < JT:
                        e_pend.append(s_exp(j + 2))
                    e_cur = e_pend.pop(0)
                    first, last = (j == 0), (j == JT - 1)
                    for co in range(CT):
                        nc.tensor.matmul(o_ps[co],
                                         vt_big[:, j, co * 128:(co + 1) * 128],
                                         e_cur, start=first, stop=last)
                    if first:
                        nc.vector.tensor_copy(esum, e_cur)
                    else:
                        nc.vector.tensor_add(esum, esum.bitcast(F32), e_cur)

                # O -> SBUF (rounds to bf16); split ACT/DVE to halve latency
                o_sb = []
                for co in range(CT):
                    o_t = opool.tile([128, 512], BF16, name=f"o_sb{co}",
                                     tag=f"ob{co}")
                    if co % 2 == 0:
                        nc.scalar.copy(out=o_t, in_=o_ps[co])
                    else:
                        nc.vector.tensor_copy(o_t, o_ps[co])
                    o_sb.append(o_t)

                # proj matmuls for block qs; alternate PSUM banks so block
                # qs+1's matmuls never wait on the ACT read of block qs
                def proj_mms(qs):
                    if qs % 2 == 1:
                        y_ps = psD.tile([128, C], F32, name=f"y_psd{qs}",
                                        tag="d")
                    else:
                        y_ps = psY.tile([128, C], F32, name=f"y_ps{qs}",
                                        tag="y")
                    for c in range(CT):
                        nc.tensor.matmul(y_ps,
                                         o_sb[c][:, qs * 128:(qs + 1) * 128],
                                         wp[c], start=(c == 0),
                                         stop=(c == CT - 1))
                    return y_ps

                # block 0's proj runs on the PE while DVE finishes the esum
                # chain for the denominator
                y_ps0 = proj_mms(0)

                # denominator: ONE matmul per q-chunk over the DVE-held esum,
                # then per-query reciprocal [128,1] per 128-row block.
                # (d_sb's ACT copy must be emitted before any y1 activation:
                # ACT is strict FIFO and y1 waits on rc which waits on d_sb.)
                d_ps = psD.tile([1, 512], F32, tag="d")
                nc.tensor.matmul(d_ps, ones_col, esum, start=True, stop=True)
                d_sb = dpool.tile([1, 512], F32, tag="dsb")
                nc.scalar.copy(out=d_sb, in_=d_ps)
                rc = []
                for qs in range(4):
                    dt_ps = psD.tile([128, 1], F32, name=f"dt_ps{qs}", tag="d")
                    nc.tensor.transpose(dt_ps,
                                        d_sb[0:1, qs * 128:(qs + 1) * 128],
                                        one1)
                    rc_t = dpool.tile([128, 1], F32, name=f"rc_{qs}",
                                      tag=f"rc{qs}")
                    nc.vector.reciprocal(out=rc_t, in_=dt_ps)
                    rc.append(rc_t)

                # residual add + 1/denom scale + store per block
                for qs in range(4):
                    y_ps = y_ps0 if qs == 0 else proj_mms(qs)
                    row0 = qc * 512 + qs * 128
                    xb_sb = xbpool.tile([128, C], F32, tag="xb")
                    nc.sync.dma_start(out=xb_sb, in_=xb_t[row0:row0 + 128, :])
                    # residual + projected v-bias row (off critical path)
                    xb2 = xbpool.tile([128, C], F32, tag="xb2")
                    nc.vector.tensor_add(xb2, xb_sb, bvp_sb)
                    y1 = ypool.tile([128, C], F32, tag="y1")
                    nc.scalar.activation(out=y1, in_=y_ps, func=AF.Identity,
                                         bias=0.0, scale=rc[qs])
                    yo = ypool.tile([128, C], F32, tag="yo")
                    nc.vector.tensor_add(yo, y1, xb2)
                    nc.sync.dma_start(out=y_t[row0:row0 + 128, :], in_=yo)

            # keep the PE activity monitor warm through the final proj
            # drains (the tail otherwise idles >3.4us and the clock halves)
            emit_burst(psS, None, 8, "cool", pstag="s", pbufs=2)

    nc.compile()
    return nc


def _get_prog():
    global _PROG
    if _PROG is None:
        _PROG = _build_program()
    return _PROG


def kernel(x, gamma, beta, w_qkv, b_qkv, w_proj, b_proj):
    import ml_dtypes
    from concourse.bass_utils import run_bass_kernel_spmd

    BF = ml_dtypes.bfloat16
    x = np.asarray(x, dtype=np.float32)
    gamma = np.asarray(gamma, dtype=np.float32)
    beta = np.asarray(beta, dtype=np.float32)
    w_qkv = np.asarray(w_qkv, dtype=np.float32)
    b_qkv = np.asarray(b_qkv, dtype=np.float32)
    w_proj = np.asarray(w_proj, dtype=np.float32)
    b_proj = np.asarray(b_proj, dtype=np.float32)

    gm = (np.arange(128)[:, None] // GSIZE ==
          np.arange(8)[None, :]).astype(np.float32)
    shared = {
        "w_qT": np.ascontiguousarray(w_qkv[0:C].T).astype(BF),
        "w_kT": np.ascontiguousarray(w_qkv[C:2 * C].T).astype(BF),
        "w_vT": np.ascontiguousarray(w_qkv[2 * C:3 * C].T).astype(BF),
        "w_pT": np.ascontiguousarray(w_proj.T).astype(BF),
        "b_v": b_qkv[2 * C:3 * C].reshape(1, C).astype(np.float32),
        "cols": np.stack([gamma.reshape(CT, 128),
                          beta.reshape(CT, 128),
                          b_qkv[0:C].reshape(CT, 128)],
                         axis=2).transpose(1, 0, 2).reshape(128, 3 * CT)
                 .astype(np.float32),
        "gmat": gm.astype(BF),
        "gmat_f": gm,
        "gmat_t": np.ascontiguousarray(gm.T),
    }

    in_maps = []
    for i in range(NCORES):
        b, h = i // 2, i % 2
        x2 = x[b].reshape(C, N)
        if h == 0:
            x_cn = x2.astype(BF)
        else:
            x_cn = np.concatenate([x2[:, NQ:], x2[:, :NQ]],
                                  axis=1).astype(BF)
        xb = np.ascontiguousarray(x2.T[h * NQ:(h + 1) * NQ] + b_proj[None, :])
        m = {"x_cn": x_cn, "xb_t": xb}
        m.update(shared)
        in_maps.append(m)

    nc = _get_prog()
    trace = os.environ.get("KERNEL_TRACE", "0") == "1"
    try:
        res = run_bass_kernel_spmd(nc, in_maps, list(range(NCORES)),
                                   trace=trace)
    except Exception:
        # transient NRT failures (e.g. a wedged core) usually clear on retry
        import time
        time.sleep(5)
        res = run_bass_kernel_spmd(nc, in_maps, list(range(NCORES)),
                                   trace=trace)
    if trace:
        kernel.last_exec_time_ns = res.exec_time_ns
        kernel.last_results = res

    out = np.empty((B, C, N), dtype=np.float32)
    for i in range(NCORES):
        b, h = i // 2, i % 2
        out[b][:, h * NQ:(h + 1) * NQ] = res.results[i]["y_t"].T
    return out.reshape(B, C, HH, WW)
